# revision 25
# baseline (speedup 1.0000x reference)
"""GNN message-passing (std aggregator) on 8 TRN2 NeuronCores.

Math per target node: count, S1 = sum x[src], S2 = sum x[src]^2;
mean = S1/max(count,eps); var = S2/count - mean^2;
std = sqrt(max(var,0)), zeroed where count <= 1.

Strategy: shard TARGET nodes across cores (no collectives). Host packs nodes
into 128-bin blocks balanced by in-degree (serpentine deal), buckets edges by
(block, src-half) with uniform tile capacity th per (block,half) so one NEFF
serves all cores. Source rows are addressed with the int16 OFFSET trick:
gather base = table + (32768 + h*50000) rows, idx in [-32768, 17231], so each
half covers 50000 rows (vs 25000 with unsigned int16), cutting slot padding
from 25% to 13%. Host pre-packs a [N,128] bf16 table xpack = [x | x^2]
(256B gather rows) and per-node rz = (count>1)/max(count,eps).
Device per core, per group of GB blocks:
  - 2x dma_gather on alternating swdge queue pairs ({0,1} / {2,3} by group
    parity) so all 4 Q7 core pairs generate descriptors concurrently,
  - DVE builds 4-wide one-hot tiles (slot-vs-iota is_equal),
  - PE matmul-accumulates [128 bins x 128] = [S1 | S2] in PSUM,
  - ACT applies rz while copying PSUM->SBUF, DVE batched var, ACT sqrt,
    one DMA out per group.
"""

import numpy as np
import ml_dtypes

N_NODES = 100000
N_FEAT = 64
N_EDGES = 1600000
P = 128
NCORES = 8
NB = 98                 # blocks per core
NBLK = NCORES * NB      # 784
GB = 7                  # blocks per group; 98 = 14*7
NHALF = 2
NH = N_NODES // NHALF   # 50000 rows per half (int16 offset trick)
IOFF = 32768            # gather base offset rows
TROWS = IOFF + NH + 32768  # padded table rows: max addr = 32768+50000+32767
EPS = 1e-8
BF16 = ml_dtypes.bfloat16

_CACHE = {}


def _build_program(f, nb, th, gb, nh):
    import concourse.bass as bass
    import concourse.bacc as bacc
    import concourse.mybir as mybir
    import concourse.tile as tile

    F32 = mybir.dt.float32
    I16 = mybir.dt.int16
    BF = mybir.dt.bfloat16
    AO = mybir.AluOpType

    w = 2 * f                  # 128 = [x | x^2]
    t = NHALF * th             # tiles (columns of 128 edges) per block
    C = nb * t                 # total 128-edge packets per core
    gcols = gb * t             # packets per group
    qcols = gb * th            # packets per (group, half)
    ng = nb // gb
    nidx = qcols * P           # indices per gather
    i16c = nidx // 16          # idx16 cols per gather

    nc = bacc.Bacc(num_swdge_queues=4)
    xd = nc.declare_dram_parameter("xpack", [TROWS, w], BF, isOutput=False)
    gidxd = nc.declare_dram_parameter(
        "gidx", [P, ng * NHALF * i16c], I16, isOutput=False)
    tgtd = nc.declare_dram_parameter("tgt", [P, C], F32, isOutput=False)
    rzd = nc.declare_dram_parameter("rz", [P, nb], F32, isOutput=False)
    outd = nc.declare_dram_parameter("out", [ng * P, gb * f], F32,
                                     isOutput=True)

    with tile.TileContext(nc) as tc:
        with (
            tc.tile_pool(name="const", bufs=1) as constp,
            tc.tile_pool(name="msg", bufs=4) as msgp,
            tc.tile_pool(name="oh", bufs=14) as ohp,
            tc.tile_pool(name="fin", bufs=2) as finp,
            tc.tile_pool(name="ov", bufs=2) as ovp,
            tc.tile_pool(name="ps", bufs=8, space="PSUM") as psump,
        ):
            # 8-wide iota [128, 8*128]: value = column index % 128
            iota8 = constp.tile([P, 8 * P], F32)
            nc.gpsimd.iota(iota8[:], pattern=[[0, 8], [1, P]], base=0,
                           channel_multiplier=0,
                           allow_small_or_imprecise_dtypes=True)
            rz = constp.tile([P, nb], F32)
            nc.sync.dma_start(out=rz[:], in_=rzd[:, :])
            # preload ALL idx/tgt streams so gathers never wait on DMA;
            # group 0's idx lands first so the first gather starts early
            gi0 = NHALF * i16c
            idxall = constp.tile([P, ng * NHALF * i16c], I16)
            nc.sync.dma_start(out=idxall[:, 0:gi0], in_=gidxd[:, 0:gi0])
            nc.sync.dma_start(out=idxall[:, gi0:], in_=gidxd[:, gi0:])
            tgall = constp.tile([P, C], F32)
            nc.sync.dma_start(out=tgall[:, 0:gcols], in_=tgtd[:, 0:gcols])
            nc.sync.dma_start(out=tgall[:, gcols:], in_=tgtd[:, gcols:])

            for g in range(ng):
                idx = idxall[:, g * NHALF * i16c:(g + 1) * NHALF * i16c]
                tg = tgall[:, g * gcols:(g + 1) * gcols]

                # 4 sub-gathers per group, emitted adjacently on queues 0-3
                # so all 4 Q7 core pairs generate descriptors concurrently.
                # Each half splits at a segment boundary (blocks 0-3 | 4-6);
                # sub order alternates by group parity to balance queue load.
                sqx = msgp.tile([P, gcols * w], BF, tag="sqx")
                s3 = sqx[:].rearrange("p (c e) -> p c e", e=w)
                bsplit = (gb // 2 + 1) * th        # 32 columns (blocks 0-3)
                for h in range(NHALF):
                    base = IOFF + h * nh
                    subs = [(0, bsplit), (bsplit, qcols)]
                    for sub, (c0_, c1_) in enumerate(subs):
                        sidx = (h * i16c * 16 + c0_ * P) // 16
                        eidx = (h * i16c * 16 + c1_ * P) // 16
                        nsub = (c1_ - c0_) * P
                        nc.gpsimd.dma_gather(
                            out_ap=s3[:, h * qcols + c0_:h * qcols + c1_, :],
                            in_ap=xd[base:base + 2, :],
                            idxs_ap=idx[:, sidx:eidx],
                            num_idxs=nsub,
                            num_idxs_reg=nsub,
                            elem_size=w,
                            single_packet=False,
                            queue_num=2 * h + (sub ^ (g % 2)),
                        )

                # one PSUM bank per block accumulator (start= clears the
                # whole bank's has_written bits, so banks can't be shared)
                pss = [psump.tile([P, w], F32, tag="ps",
                                  name=f"ps_{g}_{bl}")[:]
                       for bl in range(gb)]
                for pk in range((gcols + 7) // 8):
                    npk = min(8, gcols - 8 * pk)
                    oh4 = ohp.tile([P, 8 * P], BF)
                    nc.vector.tensor_tensor(
                        out=oh4[:, 0:npk * P]
                            .rearrange("p (c e) -> p c e", e=P),
                        in0=tg[:, 8 * pk:8 * pk + npk]
                            .rearrange("p (c u) -> p c u", u=1)
                            .to_broadcast([P, npk, P]),
                        in1=iota8[:, 0:npk * P]
                            .rearrange("p (c e) -> p c e", e=P),
                        op=AO.is_equal,
                    )
                    for i in range(npk):
                        cl = 8 * pk + i
                        h = cl // qcols
                        r = cl % qcols
                        bl = r // th
                        j = r % th
                        nc.tensor.matmul(
                            out=pss[bl],
                            lhsT=oh4[:, i * P:(i + 1) * P],
                            rhs=sqx[:, cl * w:(cl + 1) * w],
                            start=(h == 0 and j == 0),
                            stop=(h == NHALF - 1 and j == th - 1),
                        )

                # finishing: ACT copies PSUM->SBUF scaled by rz, then DVE
                # batched var over [P, gb*f], ACT sqrt, one DMA per group
                me = finp.tile([P, gb * w], F32, tag="me")
                m3 = me[:].rearrange("p (b e) -> p b e", e=w)
                for bl in range(gb):
                    b = g * gb + bl
                    nc.scalar.mul(
                        out=me[:, bl * w:(bl + 1) * w], in_=pss[bl],
                        mul=rz[:, b:b + 1])
                var = finp.tile([P, gb * f], F32, tag="var")
                v3 = var[:].rearrange("p (b e) -> p b e", e=f)
                nc.vector.tensor_tensor(
                    out=v3[:, :, :], in0=m3[:, :, 0:f], in1=m3[:, :, 0:f],
                    op=AO.mult)
                nc.vector.tensor_tensor(
                    out=v3[:, :, :], in0=m3[:, :, f:w], in1=v3[:, :, :],
                    op=AO.subtract)
                # clamp on ACT (not DVE tensor_scalar: single-src perf-mode
                # ops grab the DVE/GpSimd shared SBUF port and block against
                # long-running gather instructions holding it)
                std = ovp.tile([P, gb * f], F32, tag="std")
                nc.scalar.activation(out=var[:], in_=var[:],
                                     func=AF.Relu)
                nc.scalar.sqrt(out=std[:], in_=var[:])
                nc.sync.dma_start(
                    out=outd[g * P:(g + 1) * P, :], in_=std[:])
    return nc


def _pack_blocks(c0, c1):
    """Assign nodes to NBLK blocks of <=128 slots, balancing BOTH per-half
    in-edge sums toward <= 8*128 = 1024 (so the half tile capacity th is 8).
    Greedy on descending total degree, then swap refinement."""
    cap = 8 * P
    tot = c0 + c1
    order = np.argsort(-tot, kind="stable")
    l0 = np.zeros(NBLK)
    l1 = np.zeros(NBLK)
    ns = np.zeros(NBLK, np.int64)
    assign = np.empty(N_NODES, np.int64)
    for n in order:
        cost = np.maximum(l0 + c0[n], l1 + c1[n]) + 1e-3 * (l0 + l1)
        cost[ns >= P] = 1e18
        b = int(np.argmin(cost))
        assign[n] = b
        l0[b] += c0[n]
        l1[b] += c1[n]
        ns[b] += 1
    rng = np.random.default_rng(0)
    for _ in range(5000):
        over = np.maximum(l0 - cap, 0) + np.maximum(l1 - cap, 0)
        if over.sum() == 0:
            break
        b = int(np.argmax(over))
        half = 0 if l0[b] - cap >= l1[b] - cap else 1
        cb = c0 if half == 0 else c1
        members = np.nonzero(assign == b)[0]
        done = False
        for a in members[np.argsort(-cb[members])][:30]:
            cand = rng.integers(0, N_NODES, 8000)
            d = assign[cand]
            ok = ((l0[b] - c0[a] + c0[cand] <= cap)
                  & (l1[b] - c1[a] + c1[cand] <= cap)
                  & (l0[d] - c0[cand] + c0[a] <= cap)
                  & (l1[d] - c1[cand] + c1[a] <= cap) & (d != b))
            w = np.nonzero(ok)[0]
            if w.size:
                v = int(cand[w[0]])
                dd = assign[v]
                assign[a] = dd
                assign[v] = b
                l0[b] += c0[v] - c0[a]
                l1[b] += c1[v] - c1[a]
                l0[dd] += c0[a] - c0[v]
                l1[dd] += c1[a] - c1[v]
                done = True
                break
        if not done:
            break   # refinement stuck; th falls back to data max
    return assign


def _host_prep(x, edge_index):
    src = np.asarray(edge_index[0], dtype=np.int64)
    tgt = np.asarray(edge_index[1], dtype=np.int64)
    n_edges = src.shape[0]
    counts = np.bincount(tgt, minlength=N_NODES)

    c0 = np.bincount(tgt[src < NH], minlength=N_NODES)
    c1 = np.bincount(tgt[src >= NH], minlength=N_NODES)
    blk = _pack_blocks(c0, c1)
    # slot = index within block (stable by node id)
    order_b = np.argsort(blk, kind="stable")
    slot = np.empty(N_NODES, np.int64)
    bsort = blk[order_b]
    bstarts = np.zeros(NBLK, np.int64)
    np.cumsum(np.bincount(blk, minlength=NBLK)[:-1], out=bstarts[1:])
    slot[order_b] = np.arange(N_NODES) - bstarts[bsort]
    assert slot.max() < P

    eb = blk[tgt]                      # edge -> block
    eh = src // NH                     # edge -> src half
    es = slot[tgt]                     # edge -> slot in block
    seg = eb * NHALF + eh              # edge -> (block, half) segment
    segsums = np.bincount(seg, minlength=NBLK * NHALF)
    th = int(np.ceil(segsums.max() / P))
    cap = th * P

    # within each segment, order edges by src row for DRAM gather locality
    order_e = np.lexsort((src, seg))
    segs = seg[order_e]
    starts = np.zeros(NBLK * NHALF, np.int64)
    np.cumsum(segsums[:-1], out=starts[1:])
    within = np.arange(n_edges) - starts[segs]
    flat = segs * cap + within

    # idx values use the int16 offset trick: row r of half -> r - 32768;
    # padding slots use 0 (a valid row; one-hot column is all-zero)
    gidxq = np.zeros((NBLK, NHALF, cap), np.int16)
    tgtq = np.full((NBLK, NHALF, cap), -1.0, np.float32)
    gidxq.reshape(-1)[flat] = (src[order_e] % NH - IOFF).astype(np.int16)
    tgtq.reshape(-1)[flat] = es[order_e].astype(np.float32)

    # trailing-pop guard: the gather ucode drops trailing negative idxs from
    # each stream; ensure the final slot of every (core, group, half) stream
    # (= last block of the group, tile th-1, pos 127) has idx >= 0 by
    # swapping within its segment (edges may occupy any slot of their seg).
    ng = NB // GB
    for c in range(NCORES):
        for g in range(ng):
            # each (half, sub-gather) stream ends at a segment boundary:
            # sub 1 after block GB//2, sub 2 after block GB-1
            for bl_end in (GB // 2, GB - 1):
                b = c * NB + g * GB + bl_end
                for h in range(NHALF):
                    if gidxq[b, h, cap - 1] < 0 and tgtq[b, h, cap - 1] >= 0:
                        cand = np.nonzero(gidxq[b, h] >= 0)[0]
                        assert cand.size > 0, "no swap partner for pop guard"
                        jj = cand[0]
                        gidxq[b, h, cap - 1], gidxq[b, h, jj] = (
                            gidxq[b, h, jj], gidxq[b, h, cap - 1])
                        tgtq[b, h, cap - 1], tgtq[b, h, jj] = (
                            tgtq[b, h, jj], tgtq[b, h, cap - 1])

    # packed per-node table [x | x^2] in bf16 (256B rows), padded for the
    # offset addressing window
    xf = np.asarray(x, dtype=np.float32)
    xpack = np.zeros((TROWS, 2 * N_FEAT), BF16)
    xpack[:N_NODES, :N_FEAT] = xf.astype(BF16)
    xpack[:N_NODES, N_FEAT:] = (xf * xf).astype(BF16)
    xpack = np.ascontiguousarray(xpack)

    # per-node (count>1)/max(count,eps), laid out [slot, block] per core
    rz_node = np.where(counts > 1, 1.0 / np.maximum(counts, EPS), 0.0)
    rz_node = rz_node.astype(np.float32)
    rz_all = np.zeros((NBLK, P), np.float32)
    rz_all[blk, slot] = rz_node
    rz_all = rz_all.reshape(NCORES, NB, P)

    i16c = GB * cap // 16

    in_maps = []
    for c in range(NCORES):
        tb = tgtq[c * NB:(c + 1) * NB]          # [NB, 2, cap]
        gi = gidxq[c * NB:(c + 1) * NB]
        # tgt columns: (group, half, block, tile) -> [P, C]
        tcore = (tb.reshape(ng, GB, NHALF, cap)
                 .transpose(0, 2, 1, 3)          # [ng, 2, GB, cap]
                 .reshape(ng * NHALF * GB * th, P).T)
        # idx16: per (group, half): stream of GB*cap idxs wrapped %16
        gs = (gi.reshape(ng, GB, NHALF, cap)
              .transpose(0, 2, 1, 3)             # [ng, 2, GB, cap]
              .reshape(ng * NHALF, GB * cap))    # per-gather streams
        idx16 = np.ascontiguousarray(
            np.tile(gs.reshape(ng * NHALF, i16c, 16).transpose(0, 2, 1)
                    .reshape(ng * NHALF * 16, i16c)
                    .reshape(ng * NHALF, 16, i16c)
                    .transpose(1, 0, 2).reshape(16, ng * NHALF * i16c),
                    (8, 1)))
        in_maps.append({
            "xpack": xpack,
            "gidx": idx16,
            "tgt": np.ascontiguousarray(tcore),
            "rz": np.ascontiguousarray(rz_all[c].T),   # [P, NB]
        })
    return th, in_maps, blk, slot


def _run(x, edge_index, trace=False):
    from concourse.bass_utils import run_bass_kernel_spmd

    th, in_maps, blk, slot = _host_prep(x, edge_index)
    key = ("prog", th)
    if key not in _CACHE:
        nc_ = _build_program(N_FEAT, NB, th, GB, NH)
        nc_.finalize()
        _CACHE[key] = nc_
    nc = _CACHE[key]
    res = run_bass_kernel_spmd(
        nc, in_maps, core_ids=list(range(NCORES)), trace=trace)

    # out layout: [ng*P, GB*f]; block b = g*GB + bl lives at rows g*P + slot,
    # cols bl*f:(bl+1)*f
    out_full = np.empty((N_NODES, N_FEAT), np.float32)
    ng = NB // GB
    cores = blk // NB
    for c in range(NCORES):
        o = np.asarray(res.results[c]["out"]).reshape(ng, P, GB, N_FEAT)
        m = cores == c
        bc = blk[m] % NB
        out_full[m] = o[bc // GB, slot[m], bc % GB]
    return out_full, res


def kernel(**inputs):
    out, _ = _run(inputs["x"], inputs["edge_index"], trace=False)
    return out


# revision 27
# speedup vs baseline: 1.1596x; 1.1596x over previous
"""GNN message-passing (std aggregator) on 8 TRN2 NeuronCores.

Math per target node: count, S1 = sum x[src], S2 = sum x[src]^2;
mean = S1/max(count,eps); var = S2/count - mean^2;
std = sqrt(max(var,0)), zeroed where count <= 1.

Strategy: shard TARGET nodes across cores (no collectives). Host packs nodes
into 128-bin blocks balanced by in-degree (serpentine deal), buckets edges by
(block, src-half) with uniform tile capacity th per (block,half) so one NEFF
serves all cores. Source rows are addressed with the int16 OFFSET trick:
gather base = table + (32768 + h*50000) rows, idx in [-32768, 17231], so each
half covers 50000 rows (vs 25000 with unsigned int16), cutting slot padding
from 25% to 13%. Host pre-packs a [N,128] bf16 table xpack = [x | x^2]
(256B gather rows) and per-node rz = (count>1)/max(count,eps).
Device per core, per group of GB blocks:
  - 2x dma_gather on alternating swdge queue pairs ({0,1} / {2,3} by group
    parity) so all 4 Q7 core pairs generate descriptors concurrently,
  - DVE builds 4-wide one-hot tiles (slot-vs-iota is_equal),
  - PE matmul-accumulates [128 bins x 128] = [S1 | S2] in PSUM,
  - ACT applies rz while copying PSUM->SBUF, DVE batched var, ACT sqrt,
    one DMA out per group.
"""

import numpy as np
import ml_dtypes

N_NODES = 100000
N_FEAT = 64
N_EDGES = 1600000
P = 128
NCORES = 8
NB = 98                 # blocks per core
NBLK = NCORES * NB      # 784
GB = 7                  # blocks per group; 98 = 14*7
NHALF = 2
NH = N_NODES // NHALF   # 50000 rows per half (int16 offset trick)
IOFF = 32768            # gather base offset rows
TROWS = IOFF + NH + 32768  # padded table rows: max addr = 32768+50000+32767
EPS = 1e-8
BF16 = ml_dtypes.bfloat16

_CACHE = {}


def _build_program(f, nb, th, gb, nh):
    import concourse.bass as bass
    import concourse.bacc as bacc
    import concourse.mybir as mybir
    import concourse.tile as tile

    F32 = mybir.dt.float32
    I16 = mybir.dt.int16
    BF = mybir.dt.bfloat16
    AO = mybir.AluOpType
    AF = mybir.ActivationFunctionType

    w = 2 * f                  # 128 = [x | x^2]
    t = NHALF * th             # tiles (columns of 128 edges) per block
    C = nb * t                 # total 128-edge packets per core
    gcols = gb * t             # packets per group
    qcols = gb * th            # packets per (group, half)
    ng = nb // gb
    nidx = qcols * P           # indices per gather
    i16c = nidx // 16          # idx16 cols per gather

    nc = bacc.Bacc(num_swdge_queues=4)
    xd = nc.declare_dram_parameter("xpack", [TROWS, w], BF, isOutput=False)
    gidxd = nc.declare_dram_parameter(
        "gidx", [P, ng * NHALF * i16c], I16, isOutput=False)
    tgtd = nc.declare_dram_parameter("tgt", [P, C], F32, isOutput=False)
    rzd = nc.declare_dram_parameter("rz", [P, nb], F32, isOutput=False)
    outd = nc.declare_dram_parameter("out", [ng * P, gb * f], F32,
                                     isOutput=True)

    with tile.TileContext(nc) as tc:
        with (
            tc.tile_pool(name="const", bufs=1) as constp,
            tc.tile_pool(name="msg", bufs=4) as msgp,
            tc.tile_pool(name="oh", bufs=14) as ohp,
            tc.tile_pool(name="fin", bufs=2) as finp,
            tc.tile_pool(name="ov", bufs=2) as ovp,
            tc.tile_pool(name="ps", bufs=8, space="PSUM") as psump,
        ):
            # 8-wide iota [128, 8*128]: value = column index % 128
            iota8 = constp.tile([P, 8 * P], F32)
            nc.gpsimd.iota(iota8[:], pattern=[[0, 8], [1, P]], base=0,
                           channel_multiplier=0,
                           allow_small_or_imprecise_dtypes=True)
            rz = constp.tile([P, nb], F32)
            nc.sync.dma_start(out=rz[:], in_=rzd[:, :])
            # preload ALL idx/tgt streams so gathers never wait on DMA
            idxall = constp.tile([P, ng * NHALF * i16c], I16)
            nc.sync.dma_start(out=idxall[:], in_=gidxd[:, :])
            tgall = constp.tile([P, C], F32)
            nc.sync.dma_start(out=tgall[:], in_=tgtd[:, :])

            for g in range(ng):
                idx = idxall[:, g * NHALF * i16c:(g + 1) * NHALF * i16c]
                tg = tgall[:, g * gcols:(g + 1) * gcols]

                # 4 sub-gathers per group, emitted adjacently on queues 0-3
                # so all 4 Q7 core pairs generate descriptors concurrently.
                # Each half splits at a segment boundary (blocks 0-3 | 4-6);
                # sub order alternates by group parity to balance queue load.
                sqx = msgp.tile([P, gcols * w], BF, tag="sqx")
                s3 = sqx[:].rearrange("p (c e) -> p c e", e=w)
                bsplit = (gb // 2 + 1) * th        # 32 columns (blocks 0-3)
                for h in range(NHALF):
                    base = IOFF + h * nh
                    subs = [(0, bsplit), (bsplit, qcols)]
                    for sub, (c0_, c1_) in enumerate(subs):
                        sidx = (h * i16c * 16 + c0_ * P) // 16
                        eidx = (h * i16c * 16 + c1_ * P) // 16
                        nsub = (c1_ - c0_) * P
                        nc.gpsimd.dma_gather(
                            out_ap=s3[:, h * qcols + c0_:h * qcols + c1_, :],
                            in_ap=xd[base:base + 2, :],
                            idxs_ap=idx[:, sidx:eidx],
                            num_idxs=nsub,
                            num_idxs_reg=nsub,
                            elem_size=w,
                            single_packet=False,
                            queue_num=2 * h + (sub ^ (g % 2)),
                        )

                # one PSUM bank per block accumulator (start= clears the
                # whole bank's has_written bits, so banks can't be shared)
                pss = [psump.tile([P, w], F32, tag="ps",
                                  name=f"ps_{g}_{bl}")[:]
                       for bl in range(gb)]
                for pk in range((gcols + 7) // 8):
                    npk = min(8, gcols - 8 * pk)
                    oh4 = ohp.tile([P, 8 * P], BF)
                    nc.vector.tensor_tensor(
                        out=oh4[:, 0:npk * P]
                            .rearrange("p (c e) -> p c e", e=P),
                        in0=tg[:, 8 * pk:8 * pk + npk]
                            .rearrange("p (c u) -> p c u", u=1)
                            .to_broadcast([P, npk, P]),
                        in1=iota8[:, 0:npk * P]
                            .rearrange("p (c e) -> p c e", e=P),
                        op=AO.is_equal,
                    )
                    for i in range(npk):
                        cl = 8 * pk + i
                        h = cl // qcols
                        r = cl % qcols
                        bl = r // th
                        j = r % th
                        nc.tensor.matmul(
                            out=pss[bl],
                            lhsT=oh4[:, i * P:(i + 1) * P],
                            rhs=sqx[:, cl * w:(cl + 1) * w],
                            start=(h == 0 and j == 0),
                            stop=(h == NHALF - 1 and j == th - 1),
                        )

                # finishing: ACT copies PSUM->SBUF scaled by rz, then DVE
                # batched var over [P, gb*f], ACT sqrt, one DMA per group
                me = finp.tile([P, gb * w], F32, tag="me")
                m3 = me[:].rearrange("p (b e) -> p b e", e=w)
                for bl in range(gb):
                    b = g * gb + bl
                    nc.scalar.mul(
                        out=me[:, bl * w:(bl + 1) * w], in_=pss[bl],
                        mul=rz[:, b:b + 1])
                var = finp.tile([P, gb * f], F32, tag="var")
                v3 = var[:].rearrange("p (b e) -> p b e", e=f)
                nc.vector.tensor_tensor(
                    out=v3[:, :, :], in0=m3[:, :, 0:f], in1=m3[:, :, 0:f],
                    op=AO.mult)
                nc.vector.tensor_tensor(
                    out=v3[:, :, :], in0=m3[:, :, f:w], in1=v3[:, :, :],
                    op=AO.subtract)
                # clamp on ACT (not DVE tensor_scalar: single-src perf-mode
                # ops grab the DVE/GpSimd shared SBUF port and block against
                # long-running gather instructions holding it)
                std = ovp.tile([P, gb * f], F32, tag="std")
                nc.scalar.activation(out=var[:], in_=var[:],
                                     func=AF.Relu)
                nc.scalar.sqrt(out=std[:], in_=var[:])
                nc.sync.dma_start(
                    out=outd[g * P:(g + 1) * P, :], in_=std[:])
    return nc


def _pack_blocks(c0, c1):
    """Assign nodes to NBLK blocks of <=128 slots, balancing BOTH per-half
    in-edge sums toward <= 8*128 = 1024 (so the half tile capacity th is 8).
    Greedy on descending total degree, then swap refinement."""
    cap = 8 * P
    tot = c0 + c1
    order = np.argsort(-tot, kind="stable")
    l0 = np.zeros(NBLK)
    l1 = np.zeros(NBLK)
    ns = np.zeros(NBLK, np.int64)
    assign = np.empty(N_NODES, np.int64)
    for n in order:
        cost = np.maximum(l0 + c0[n], l1 + c1[n]) + 1e-3 * (l0 + l1)
        cost[ns >= P] = 1e18
        b = int(np.argmin(cost))
        assign[n] = b
        l0[b] += c0[n]
        l1[b] += c1[n]
        ns[b] += 1
    rng = np.random.default_rng(0)
    for _ in range(5000):
        over = np.maximum(l0 - cap, 0) + np.maximum(l1 - cap, 0)
        if over.sum() == 0:
            break
        b = int(np.argmax(over))
        half = 0 if l0[b] - cap >= l1[b] - cap else 1
        cb = c0 if half == 0 else c1
        members = np.nonzero(assign == b)[0]
        done = False
        for a in members[np.argsort(-cb[members])][:30]:
            cand = rng.integers(0, N_NODES, 8000)
            d = assign[cand]
            ok = ((l0[b] - c0[a] + c0[cand] <= cap)
                  & (l1[b] - c1[a] + c1[cand] <= cap)
                  & (l0[d] - c0[cand] + c0[a] <= cap)
                  & (l1[d] - c1[cand] + c1[a] <= cap) & (d != b))
            w = np.nonzero(ok)[0]
            if w.size:
                v = int(cand[w[0]])
                dd = assign[v]
                assign[a] = dd
                assign[v] = b
                l0[b] += c0[v] - c0[a]
                l1[b] += c1[v] - c1[a]
                l0[dd] += c0[a] - c0[v]
                l1[dd] += c1[a] - c1[v]
                done = True
                break
        if not done:
            break   # refinement stuck; th falls back to data max
    return assign


def _host_prep(x, edge_index):
    src = np.asarray(edge_index[0], dtype=np.int64)
    tgt = np.asarray(edge_index[1], dtype=np.int64)
    n_edges = src.shape[0]
    counts = np.bincount(tgt, minlength=N_NODES)

    c0 = np.bincount(tgt[src < NH], minlength=N_NODES)
    c1 = np.bincount(tgt[src >= NH], minlength=N_NODES)
    blk = _pack_blocks(c0, c1)
    # slot = index within block (stable by node id)
    order_b = np.argsort(blk, kind="stable")
    slot = np.empty(N_NODES, np.int64)
    bsort = blk[order_b]
    bstarts = np.zeros(NBLK, np.int64)
    np.cumsum(np.bincount(blk, minlength=NBLK)[:-1], out=bstarts[1:])
    slot[order_b] = np.arange(N_NODES) - bstarts[bsort]
    assert slot.max() < P

    eb = blk[tgt]                      # edge -> block
    eh = src // NH                     # edge -> src half
    es = slot[tgt]                     # edge -> slot in block
    seg = eb * NHALF + eh              # edge -> (block, half) segment
    segsums = np.bincount(seg, minlength=NBLK * NHALF)
    th = int(np.ceil(segsums.max() / P))
    cap = th * P

    # within each segment, order edges by src row for DRAM gather locality
    order_e = np.lexsort((src, seg))
    segs = seg[order_e]
    starts = np.zeros(NBLK * NHALF, np.int64)
    np.cumsum(segsums[:-1], out=starts[1:])
    within = np.arange(n_edges) - starts[segs]
    flat = segs * cap + within

    # idx values use the int16 offset trick: row r of half -> r - 32768;
    # padding slots use 0 (a valid row; one-hot column is all-zero)
    gidxq = np.zeros((NBLK, NHALF, cap), np.int16)
    tgtq = np.full((NBLK, NHALF, cap), -1.0, np.float32)
    gidxq.reshape(-1)[flat] = (src[order_e] % NH - IOFF).astype(np.int16)
    tgtq.reshape(-1)[flat] = es[order_e].astype(np.float32)

    # trailing-pop guard: the gather ucode drops trailing negative idxs from
    # each stream; ensure the final slot of every (core, group, half) stream
    # (= last block of the group, tile th-1, pos 127) has idx >= 0 by
    # swapping within its segment (edges may occupy any slot of their seg).
    ng = NB // GB
    for c in range(NCORES):
        for g in range(ng):
            # each (half, sub-gather) stream ends at a segment boundary:
            # sub 1 after block GB//2, sub 2 after block GB-1
            for bl_end in (GB // 2, GB - 1):
                b = c * NB + g * GB + bl_end
                for h in range(NHALF):
                    if gidxq[b, h, cap - 1] < 0 and tgtq[b, h, cap - 1] >= 0:
                        cand = np.nonzero(gidxq[b, h] >= 0)[0]
                        assert cand.size > 0, "no swap partner for pop guard"
                        jj = cand[0]
                        gidxq[b, h, cap - 1], gidxq[b, h, jj] = (
                            gidxq[b, h, jj], gidxq[b, h, cap - 1])
                        tgtq[b, h, cap - 1], tgtq[b, h, jj] = (
                            tgtq[b, h, jj], tgtq[b, h, cap - 1])

    # packed per-node table [x | x^2] in bf16 (256B rows), padded for the
    # offset addressing window
    xf = np.asarray(x, dtype=np.float32)
    xpack = np.zeros((TROWS, 2 * N_FEAT), BF16)
    xpack[:N_NODES, :N_FEAT] = xf.astype(BF16)
    xpack[:N_NODES, N_FEAT:] = (xf * xf).astype(BF16)
    xpack = np.ascontiguousarray(xpack)

    # per-node (count>1)/max(count,eps), laid out [slot, block] per core
    rz_node = np.where(counts > 1, 1.0 / np.maximum(counts, EPS), 0.0)
    rz_node = rz_node.astype(np.float32)
    rz_all = np.zeros((NBLK, P), np.float32)
    rz_all[blk, slot] = rz_node
    rz_all = rz_all.reshape(NCORES, NB, P)

    i16c = GB * cap // 16

    in_maps = []
    for c in range(NCORES):
        tb = tgtq[c * NB:(c + 1) * NB]          # [NB, 2, cap]
        gi = gidxq[c * NB:(c + 1) * NB]
        # tgt columns: (group, half, block, tile) -> [P, C]
        tcore = (tb.reshape(ng, GB, NHALF, cap)
                 .transpose(0, 2, 1, 3)          # [ng, 2, GB, cap]
                 .reshape(ng * NHALF * GB * th, P).T)
        # idx16: per (group, half): stream of GB*cap idxs wrapped %16
        gs = (gi.reshape(ng, GB, NHALF, cap)
              .transpose(0, 2, 1, 3)             # [ng, 2, GB, cap]
              .reshape(ng * NHALF, GB * cap))    # per-gather streams
        idx16 = np.ascontiguousarray(
            np.tile(gs.reshape(ng * NHALF, i16c, 16).transpose(0, 2, 1)
                    .reshape(ng * NHALF * 16, i16c)
                    .reshape(ng * NHALF, 16, i16c)
                    .transpose(1, 0, 2).reshape(16, ng * NHALF * i16c),
                    (8, 1)))
        in_maps.append({
            "xpack": xpack,
            "gidx": idx16,
            "tgt": np.ascontiguousarray(tcore),
            "rz": np.ascontiguousarray(rz_all[c].T),   # [P, NB]
        })
    return th, in_maps, blk, slot


def _run(x, edge_index, trace=False):
    from concourse.bass_utils import run_bass_kernel_spmd

    th, in_maps, blk, slot = _host_prep(x, edge_index)
    key = ("prog", th)
    if key not in _CACHE:
        nc_ = _build_program(N_FEAT, NB, th, GB, NH)
        nc_.finalize()
        _CACHE[key] = nc_
    nc = _CACHE[key]
    res = run_bass_kernel_spmd(
        nc, in_maps, core_ids=list(range(NCORES)), trace=trace)

    # out layout: [ng*P, GB*f]; block b = g*GB + bl lives at rows g*P + slot,
    # cols bl*f:(bl+1)*f
    out_full = np.empty((N_NODES, N_FEAT), np.float32)
    ng = NB // GB
    cores = blk // NB
    for c in range(NCORES):
        o = np.asarray(res.results[c]["out"]).reshape(ng, P, GB, N_FEAT)
        m = cores == c
        bc = blk[m] % NB
        out_full[m] = o[bc // GB, slot[m], bc % GB]
    return out_full, res


def kernel(**inputs):
    out, _ = _run(inputs["x"], inputs["edge_index"], trace=False)
    return out


# revision 30
# speedup vs baseline: 1.1881x; 1.0246x over previous
"""GNN message-passing (std aggregator) on 8 TRN2 NeuronCores.

Math per target node: count, S1 = sum x[src], S2 = sum x[src]^2;
mean = S1/max(count,eps); var = S2/count - mean^2;
std = sqrt(max(var,0)), zeroed where count <= 1.

Strategy: shard TARGET nodes across cores (no collectives). Host packs nodes
into 128-bin blocks balanced by in-degree (serpentine deal), buckets edges by
(block, src-half) with uniform tile capacity th per (block,half) so one NEFF
serves all cores. Source rows are addressed with the int16 OFFSET trick:
gather base = table + (32768 + h*50000) rows, idx in [-32768, 17231], so each
half covers 50000 rows (vs 25000 with unsigned int16), cutting slot padding
from 25% to 13%. Host pre-packs a [N,128] bf16 table xpack = [x | x^2]
(256B gather rows) and per-node rz = (count>1)/max(count,eps).
Device per core, per group of GB blocks:
  - 2x dma_gather on alternating swdge queue pairs ({0,1} / {2,3} by group
    parity) so all 4 Q7 core pairs generate descriptors concurrently,
  - DVE builds 4-wide one-hot tiles (slot-vs-iota is_equal),
  - PE matmul-accumulates [128 bins x 128] = [S1 | S2] in PSUM,
  - ACT applies rz while copying PSUM->SBUF, DVE batched var, ACT sqrt,
    one DMA out per group.
"""

import numpy as np
import ml_dtypes

N_NODES = 100000
N_FEAT = 64
N_EDGES = 1600000
P = 128
NCORES = 8
NB = 98                 # blocks per core
NBLK = NCORES * NB      # 784
GB = 7                  # blocks per group; 98 = 14*7
NHALF = 2
NH = N_NODES // NHALF   # 50000 rows per half (int16 offset trick)
IOFF = 32768            # gather base offset rows
TROWS = IOFF + NH + 32768  # padded table rows: max addr = 32768+50000+32767
EPS = 1e-8
BF16 = ml_dtypes.bfloat16

_CACHE = {}


def _build_program(f, nb, th, gb, nh):
    import concourse.bass as bass
    import concourse.bacc as bacc
    import concourse.mybir as mybir
    import concourse.tile as tile

    F32 = mybir.dt.float32
    I16 = mybir.dt.int16
    BF = mybir.dt.bfloat16
    AO = mybir.AluOpType
    AF = mybir.ActivationFunctionType

    w = 2 * f                  # 128 = [x | x^2]
    t = NHALF * th             # tiles (columns of 128 edges) per block
    C = nb * t                 # total 128-edge packets per core
    gcols = gb * t             # packets per group
    qcols = gb * th            # packets per (group, half)
    ng = nb // gb
    nidx = qcols * P           # indices per gather
    i16c = nidx // 16          # idx16 cols per gather

    nc = bacc.Bacc(num_swdge_queues=4)
    xd = nc.declare_dram_parameter("xpack", [TROWS, w], BF, isOutput=False)
    gidxd = nc.declare_dram_parameter(
        "gidx", [P, ng * NHALF * i16c], I16, isOutput=False)
    tgtd = nc.declare_dram_parameter("tgt", [P, C], F32, isOutput=False)
    rzd = nc.declare_dram_parameter("rz", [P, nb], F32, isOutput=False)
    outd = nc.declare_dram_parameter("out", [ng * P, gb * f], F32,
                                     isOutput=True)

    with tile.TileContext(nc) as tc:
        with (
            tc.tile_pool(name="const", bufs=1) as constp,
            tc.tile_pool(name="msg", bufs=4) as msgp,
            tc.tile_pool(name="oh", bufs=14) as ohp,
            tc.tile_pool(name="fin", bufs=2) as finp,
            tc.tile_pool(name="ov", bufs=2) as ovp,
            tc.tile_pool(name="ps", bufs=8, space="PSUM") as psump,
        ):
            # 8-wide iota [128, 8*128]: value = column index % 128
            iota8 = constp.tile([P, 8 * P], F32)
            nc.gpsimd.iota(iota8[:], pattern=[[0, 8], [1, P]], base=0,
                           channel_multiplier=0,
                           allow_small_or_imprecise_dtypes=True)
            rz = constp.tile([P, nb], F32)
            nc.sync.dma_start(out=rz[:], in_=rzd[:, :])
            # preload ALL idx/tgt streams so gathers never wait on DMA
            idxall = constp.tile([P, ng * NHALF * i16c], I16)
            nc.sync.dma_start(out=idxall[:], in_=gidxd[:, :])
            tgall = constp.tile([P, C], F32)
            nc.sync.dma_start(out=tgall[:], in_=tgtd[:, :])

            for g in range(ng):
                idx = idxall[:, g * NHALF * i16c:(g + 1) * NHALF * i16c]
                tg = tgall[:, g * gcols:(g + 1) * gcols]

                # 4 sub-gathers per group, emitted adjacently on queues 0-3
                # so all 4 Q7 core pairs generate descriptors concurrently.
                # Each half splits at a segment boundary (blocks 0-3 | 4-6);
                # sub order alternates by group parity to balance queue load.
                sqx = msgp.tile([P, gcols * w], BF, tag="sqx")
                s3 = sqx[:].rearrange("p (c e) -> p c e", e=w)
                bsplit = qcols // 2    # equal sub sizes -> no retire bubbles
                for h in range(NHALF):
                    base = IOFF + h * nh
                    subs = [(0, bsplit), (bsplit, qcols)]
                    for sub, (c0_, c1_) in enumerate(subs):
                        sidx = (h * i16c * 16 + c0_ * P) // 16
                        eidx = (h * i16c * 16 + c1_ * P) // 16
                        nsub = (c1_ - c0_) * P
                        nc.gpsimd.dma_gather(
                            out_ap=s3[:, h * qcols + c0_:h * qcols + c1_, :],
                            in_ap=xd[base:base + 2, :],
                            idxs_ap=idx[:, sidx:eidx],
                            num_idxs=nsub,
                            num_idxs_reg=nsub,
                            elem_size=w,
                            single_packet=False,
                            queue_num=2 * h + (sub ^ (g % 2)),
                        )

                # one PSUM bank per block accumulator (start= clears the
                # whole bank's has_written bits, so banks can't be shared)
                pss = [psump.tile([P, w], F32, tag="ps",
                                  name=f"ps_{g}_{bl}")[:]
                       for bl in range(gb)]
                for pk in range((gcols + 7) // 8):
                    npk = min(8, gcols - 8 * pk)
                    oh4 = ohp.tile([P, 8 * P], BF)
                    nc.vector.tensor_tensor(
                        out=oh4[:, 0:npk * P]
                            .rearrange("p (c e) -> p c e", e=P),
                        in0=tg[:, 8 * pk:8 * pk + npk]
                            .rearrange("p (c u) -> p c u", u=1)
                            .to_broadcast([P, npk, P]),
                        in1=iota8[:, 0:npk * P]
                            .rearrange("p (c e) -> p c e", e=P),
                        op=AO.is_equal,
                    )
                    for i in range(npk):
                        cl = 8 * pk + i
                        h = cl // qcols
                        r = cl % qcols
                        bl = r // th
                        j = r % th
                        nc.tensor.matmul(
                            out=pss[bl],
                            lhsT=oh4[:, i * P:(i + 1) * P],
                            rhs=sqx[:, cl * w:(cl + 1) * w],
                            start=(h == 0 and j == 0),
                            stop=(h == NHALF - 1 and j == th - 1),
                        )

                # finishing: ACT copies PSUM->SBUF scaled by rz, then DVE
                # batched var over [P, gb*f], ACT sqrt, one DMA per group
                me = finp.tile([P, gb * w], F32, tag="me")
                m3 = me[:].rearrange("p (b e) -> p b e", e=w)
                for bl in range(gb):
                    b = g * gb + bl
                    nc.scalar.mul(
                        out=me[:, bl * w:(bl + 1) * w], in_=pss[bl],
                        mul=rz[:, b:b + 1])
                var = finp.tile([P, gb * f], F32, tag="var")
                v3 = var[:].rearrange("p (b e) -> p b e", e=f)
                nc.vector.tensor_tensor(
                    out=v3[:, :, :], in0=m3[:, :, 0:f], in1=m3[:, :, 0:f],
                    op=AO.mult)
                nc.vector.tensor_tensor(
                    out=v3[:, :, :], in0=m3[:, :, f:w], in1=v3[:, :, :],
                    op=AO.subtract)
                # clamp on ACT (not DVE tensor_scalar: single-src perf-mode
                # ops grab the DVE/GpSimd shared SBUF port and block against
                # long-running gather instructions holding it)
                std = ovp.tile([P, gb * f], F32, tag="std")
                nc.scalar.activation(out=var[:], in_=var[:],
                                     func=AF.Relu)
                nc.scalar.sqrt(out=std[:], in_=var[:])
                nc.sync.dma_start(
                    out=outd[g * P:(g + 1) * P, :], in_=std[:])
    return nc


def _pack_blocks(c0, c1):
    """Assign nodes to NBLK blocks of <=128 slots, balancing BOTH per-half
    in-edge sums toward <= 8*128 = 1024 (so the half tile capacity th is 8).
    Greedy on descending total degree, then swap refinement."""
    cap = 8 * P
    tot = c0 + c1
    order = np.argsort(-tot, kind="stable")
    l0 = np.zeros(NBLK)
    l1 = np.zeros(NBLK)
    ns = np.zeros(NBLK, np.int64)
    assign = np.empty(N_NODES, np.int64)
    for n in order:
        cost = np.maximum(l0 + c0[n], l1 + c1[n]) + 1e-3 * (l0 + l1)
        cost[ns >= P] = 1e18
        b = int(np.argmin(cost))
        assign[n] = b
        l0[b] += c0[n]
        l1[b] += c1[n]
        ns[b] += 1
    rng = np.random.default_rng(0)
    for _ in range(5000):
        over = np.maximum(l0 - cap, 0) + np.maximum(l1 - cap, 0)
        if over.sum() == 0:
            break
        b = int(np.argmax(over))
        half = 0 if l0[b] - cap >= l1[b] - cap else 1
        cb = c0 if half == 0 else c1
        members = np.nonzero(assign == b)[0]
        done = False
        for a in members[np.argsort(-cb[members])][:30]:
            cand = rng.integers(0, N_NODES, 8000)
            d = assign[cand]
            ok = ((l0[b] - c0[a] + c0[cand] <= cap)
                  & (l1[b] - c1[a] + c1[cand] <= cap)
                  & (l0[d] - c0[cand] + c0[a] <= cap)
                  & (l1[d] - c1[cand] + c1[a] <= cap) & (d != b))
            w = np.nonzero(ok)[0]
            if w.size:
                v = int(cand[w[0]])
                dd = assign[v]
                assign[a] = dd
                assign[v] = b
                l0[b] += c0[v] - c0[a]
                l1[b] += c1[v] - c1[a]
                l0[dd] += c0[a] - c0[v]
                l1[dd] += c1[a] - c1[v]
                done = True
                break
        if not done:
            break   # refinement stuck; th falls back to data max
    return assign


def _host_prep(x, edge_index):
    src = np.asarray(edge_index[0], dtype=np.int64)
    tgt = np.asarray(edge_index[1], dtype=np.int64)
    n_edges = src.shape[0]
    counts = np.bincount(tgt, minlength=N_NODES)

    c0 = np.bincount(tgt[src < NH], minlength=N_NODES)
    c1 = np.bincount(tgt[src >= NH], minlength=N_NODES)
    blk = _pack_blocks(c0, c1)
    # slot = index within block (stable by node id)
    order_b = np.argsort(blk, kind="stable")
    slot = np.empty(N_NODES, np.int64)
    bsort = blk[order_b]
    bstarts = np.zeros(NBLK, np.int64)
    np.cumsum(np.bincount(blk, minlength=NBLK)[:-1], out=bstarts[1:])
    slot[order_b] = np.arange(N_NODES) - bstarts[bsort]
    assert slot.max() < P

    eb = blk[tgt]                      # edge -> block
    eh = src // NH                     # edge -> src half
    es = slot[tgt]                     # edge -> slot in block
    seg = eb * NHALF + eh              # edge -> (block, half) segment
    segsums = np.bincount(seg, minlength=NBLK * NHALF)
    th = int(np.ceil(segsums.max() / P))
    cap = th * P

    # within each segment, order edges by src row (descending) for DRAM
    # gather locality; descending puts negative offset-idxs last so the
    # trailing-pop guard below can always find a swap partner
    order_e = np.lexsort((-src, seg))
    segs = seg[order_e]
    starts = np.zeros(NBLK * NHALF, np.int64)
    np.cumsum(segsums[:-1], out=starts[1:])
    within = np.arange(n_edges) - starts[segs]
    flat = segs * cap + within

    # idx values use the int16 offset trick: row r of half -> r - 32768;
    # padding slots use 0 (a valid row; one-hot column is all-zero)
    gidxq = np.zeros((NBLK, NHALF, cap), np.int16)
    tgtq = np.full((NBLK, NHALF, cap), -1.0, np.float32)
    gidxq.reshape(-1)[flat] = (src[order_e] % NH - IOFF).astype(np.int16)
    tgtq.reshape(-1)[flat] = es[order_e].astype(np.float32)

    # trailing-pop guard: the gather ucode drops trailing negative idxs from
    # each stream; ensure the final slot of every (core, group, half) stream
    # (= last block of the group, tile th-1, pos 127) has idx >= 0 by
    # swapping within its segment (edges may occupy any slot of their seg).
    # each (group, half) stream is gathered as two equal sub-gathers split at
    # column qcols//2; the ucode pops TRAILING negative idxs per sub-stream,
    # so each sub-stream's final slot must hold idx >= 0. Swap partners must
    # stay within the same segment AND at/before the boundary slot.
    ng = NB // GB
    qcols = GB * th
    for cb, cs in ((qcols // 2, 0), (qcols, qcols // 2)):
        end = cb * P - 1                    # flat slot in the (g,h) stream
        bl_end = end // cap                 # block within group
        pos = end % cap                     # position within segment
        lo = max(0, cs * P - bl_end * cap)  # earliest in-sub segment slot
        for c in range(NCORES):
            for g in range(ng):
                b = c * NB + g * GB + bl_end
                for h in range(NHALF):
                    if gidxq[b, h, pos] < 0 and tgtq[b, h, pos] >= 0:
                        cand = np.nonzero(gidxq[b, h, lo:pos + 1] >= 0)[0]
                        assert cand.size > 0, "no swap partner for pop guard"
                        jj = lo + cand[0]
                        gidxq[b, h, pos], gidxq[b, h, jj] = (
                            gidxq[b, h, jj], gidxq[b, h, pos])
                        tgtq[b, h, pos], tgtq[b, h, jj] = (
                            tgtq[b, h, jj], tgtq[b, h, pos])

    # packed per-node table [x | x^2] in bf16 (256B rows), padded for the
    # offset addressing window
    xf = np.asarray(x, dtype=np.float32)
    xpack = np.zeros((TROWS, 2 * N_FEAT), BF16)
    xpack[:N_NODES, :N_FEAT] = xf.astype(BF16)
    xpack[:N_NODES, N_FEAT:] = (xf * xf).astype(BF16)
    xpack = np.ascontiguousarray(xpack)

    # per-node (count>1)/max(count,eps), laid out [slot, block] per core
    rz_node = np.where(counts > 1, 1.0 / np.maximum(counts, EPS), 0.0)
    rz_node = rz_node.astype(np.float32)
    rz_all = np.zeros((NBLK, P), np.float32)
    rz_all[blk, slot] = rz_node
    rz_all = rz_all.reshape(NCORES, NB, P)

    i16c = GB * cap // 16

    in_maps = []
    for c in range(NCORES):
        tb = tgtq[c * NB:(c + 1) * NB]          # [NB, 2, cap]
        gi = gidxq[c * NB:(c + 1) * NB]
        # tgt columns: (group, half, block, tile) -> [P, C]
        tcore = (tb.reshape(ng, GB, NHALF, cap)
                 .transpose(0, 2, 1, 3)          # [ng, 2, GB, cap]
                 .reshape(ng * NHALF * GB * th, P).T)
        # idx16: per (group, half): stream of GB*cap idxs wrapped %16
        gs = (gi.reshape(ng, GB, NHALF, cap)
              .transpose(0, 2, 1, 3)             # [ng, 2, GB, cap]
              .reshape(ng * NHALF, GB * cap))    # per-gather streams
        idx16 = np.ascontiguousarray(
            np.tile(gs.reshape(ng * NHALF, i16c, 16).transpose(0, 2, 1)
                    .reshape(ng * NHALF * 16, i16c)
                    .reshape(ng * NHALF, 16, i16c)
                    .transpose(1, 0, 2).reshape(16, ng * NHALF * i16c),
                    (8, 1)))
        in_maps.append({
            "xpack": xpack,
            "gidx": idx16,
            "tgt": np.ascontiguousarray(tcore),
            "rz": np.ascontiguousarray(rz_all[c].T),   # [P, NB]
        })
    return th, in_maps, blk, slot


def _run(x, edge_index, trace=False):
    from concourse.bass_utils import run_bass_kernel_spmd

    th, in_maps, blk, slot = _host_prep(x, edge_index)
    key = ("prog", th)
    if key not in _CACHE:
        nc_ = _build_program(N_FEAT, NB, th, GB, NH)
        nc_.finalize()
        _CACHE[key] = nc_
    nc = _CACHE[key]
    res = run_bass_kernel_spmd(
        nc, in_maps, core_ids=list(range(NCORES)), trace=trace)

    # out layout: [ng*P, GB*f]; block b = g*GB + bl lives at rows g*P + slot,
    # cols bl*f:(bl+1)*f
    out_full = np.empty((N_NODES, N_FEAT), np.float32)
    ng = NB // GB
    cores = blk // NB
    for c in range(NCORES):
        o = np.asarray(res.results[c]["out"]).reshape(ng, P, GB, N_FEAT)
        m = cores == c
        bc = blk[m] % NB
        out_full[m] = o[bc // GB, slot[m], bc % GB]
    return out_full, res


def kernel(**inputs):
    out, _ = _run(inputs["x"], inputs["edge_index"], trace=False)
    return out


# revision 32
# speedup vs baseline: 1.2080x; 1.0168x over previous
"""GNN message-passing (std aggregator) on 8 TRN2 NeuronCores.

Math per target node: count, S1 = sum x[src], S2 = sum x[src]^2;
mean = S1/max(count,eps); var = S2/count - mean^2;
std = sqrt(max(var,0)), zeroed where count <= 1.

Strategy: shard TARGET nodes across cores (no collectives). Host packs nodes
into 128-bin blocks balancing BOTH per-half in-edge sums (greedy + swap
refinement) so the per-(block, src-half) tile capacity th hits 8 (0.4% slot
padding). Source rows are addressed with the int16 OFFSET trick: gather base
= table + (32768 + h*50000) rows, idx in [-32768, 17231], so each half
covers 50000 rows (vs 25000 with unsigned int16). Host pre-packs a [N,128]
bf16 table xpack = [x | x^2] (256B gather rows, the swdge minimum) and
per-node rz = (count>1)/max(count,eps), which folds the mean division AND
the count<=1 mask into one scalar.
Device per core, per group of GB=7 blocks:
  - 4 equal dma_gather subs on swdge queues 0-3 (each queue owns a Q7 core
    pair; descriptor generation at ~7.9ns/edge/pair is THE bottleneck, so
    all 4 pairs must run concurrently and subs must be equal-sized to avoid
    in-order-retire bubbles), pulling xpack rows straight into matmul-ready
    rhs tiles [128 edge-slots, 128] bf16,
  - DVE builds 8-wide one-hot tiles (slot-vs-iota is_equal, 1x mode; DVE
    must avoid single-src perf-mode ops which lock the shared GpSimd port),
  - PE matmul-accumulates [128 bins x 128] = [S1 | S2], one PSUM bank per
    block (start= clears the whole bank's has_written bits),
  - ACT applies rz while copying PSUM->SBUF, DVE batched var, ACT
    relu+sqrt, one DMA out per group.
"""

import numpy as np
import ml_dtypes

N_NODES = 100000
N_FEAT = 64
N_EDGES = 1600000
P = 128
NCORES = 8
NB = 98                 # blocks per core
NBLK = NCORES * NB      # 784
GB = 7                  # blocks per group; 98 = 14*7
NHALF = 2
NH = N_NODES // NHALF   # 50000 rows per half (int16 offset trick)
IOFF = 32768            # gather base offset rows
TROWS = IOFF + NH + 32768  # padded table rows: max addr = 32768+50000+32767
EPS = 1e-8
BF16 = ml_dtypes.bfloat16

_CACHE = {}


def _build_program(f, nb, th, gb, nh):
    import concourse.bass as bass
    import concourse.bacc as bacc
    import concourse.mybir as mybir
    import concourse.tile as tile

    F32 = mybir.dt.float32
    I16 = mybir.dt.int16
    BF = mybir.dt.bfloat16
    AO = mybir.AluOpType
    AF = mybir.ActivationFunctionType

    w = 2 * f                  # 128 = [x | x^2]
    t = NHALF * th             # tiles (columns of 128 edges) per block
    C = nb * t                 # total 128-edge packets per core
    gcols = gb * t             # packets per group
    qcols = gb * th            # packets per (group, half)
    ng = nb // gb
    nidx = qcols * P           # indices per gather
    i16c = nidx // 16          # idx16 cols per gather

    nc = bacc.Bacc(num_swdge_queues=4)
    xd = nc.declare_dram_parameter("xpack", [TROWS, w], BF, isOutput=False)
    gidxd = nc.declare_dram_parameter(
        "gidx", [P, ng * NHALF * i16c], I16, isOutput=False)
    tgtd = nc.declare_dram_parameter("tgt", [P, C], F32, isOutput=False)
    rzd = nc.declare_dram_parameter("rz", [P, nb], F32, isOutput=False)
    outd = nc.declare_dram_parameter("out", [ng * P, gb * f], F32,
                                     isOutput=True)

    with tile.TileContext(nc) as tc:
        with (
            tc.tile_pool(name="const", bufs=1) as constp,
            tc.tile_pool(name="msg", bufs=4) as msgp,
            tc.tile_pool(name="oh", bufs=14) as ohp,
            tc.tile_pool(name="fin", bufs=2) as finp,
            tc.tile_pool(name="ov", bufs=2) as ovp,
            tc.tile_pool(name="ps", bufs=8, space="PSUM") as psump,
        ):
            # 8-wide iota [128, 8*128]: value = column index % 128
            iota8 = constp.tile([P, 8 * P], F32)
            nc.gpsimd.iota(iota8[:], pattern=[[0, 8], [1, P]], base=0,
                           channel_multiplier=0,
                           allow_small_or_imprecise_dtypes=True)
            # preload idx/tgt streams so gathers never wait on DMA mid-run;
            # group 0's idx gets its own small tile + DMA so the first
            # gather starts without waiting for the full preload
            gi0 = NHALF * i16c
            idx0 = constp.tile([P, gi0], I16)
            nc.sync.dma_start(out=idx0[:], in_=gidxd[:, 0:gi0])
            idxall = constp.tile([P, (ng - 1) * gi0], I16)
            nc.sync.dma_start(out=idxall[:], in_=gidxd[:, gi0:])
            tgall = constp.tile([P, C], F32)
            nc.sync.dma_start(out=tgall[:], in_=tgtd[:, :])
            rz = constp.tile([P, nb], F32)
            nc.sync.dma_start(out=rz[:], in_=rzd[:, :])

            for g in range(ng):
                idx = (idx0[:] if g == 0
                       else idxall[:, (g - 1) * gi0:g * gi0])
                tg = tgall[:, g * gcols:(g + 1) * gcols]

                # 4 sub-gathers per group, emitted adjacently on queues 0-3
                # so all 4 Q7 core pairs generate descriptors concurrently.
                # Each half splits at a segment boundary (blocks 0-3 | 4-6);
                # sub order alternates by group parity to balance queue load.
                sqx = msgp.tile([P, gcols * w], BF, tag="sqx")
                s3 = sqx[:].rearrange("p (c e) -> p c e", e=w)
                bsplit = qcols // 2    # equal sub sizes -> no retire bubbles
                for h in range(NHALF):
                    base = IOFF + h * nh
                    subs = [(0, bsplit), (bsplit, qcols)]
                    for sub, (c0_, c1_) in enumerate(subs):
                        sidx = (h * i16c * 16 + c0_ * P) // 16
                        eidx = (h * i16c * 16 + c1_ * P) // 16
                        nsub = (c1_ - c0_) * P
                        nc.gpsimd.dma_gather(
                            out_ap=s3[:, h * qcols + c0_:h * qcols + c1_, :],
                            in_ap=xd[base:base + 2, :],
                            idxs_ap=idx[:, sidx:eidx],
                            num_idxs=nsub,
                            num_idxs_reg=nsub,
                            elem_size=w,
                            single_packet=False,
                            queue_num=2 * h + (sub ^ (g % 2)),
                        )

                # one PSUM bank per block accumulator (start= clears the
                # whole bank's has_written bits, so banks can't be shared)
                pss = [psump.tile([P, w], F32, tag="ps",
                                  name=f"ps_{g}_{bl}")[:]
                       for bl in range(gb)]
                for pk in range((gcols + 7) // 8):
                    npk = min(8, gcols - 8 * pk)
                    oh4 = ohp.tile([P, 8 * P], BF)
                    nc.vector.tensor_tensor(
                        out=oh4[:, 0:npk * P]
                            .rearrange("p (c e) -> p c e", e=P),
                        in0=tg[:, 8 * pk:8 * pk + npk]
                            .rearrange("p (c u) -> p c u", u=1)
                            .to_broadcast([P, npk, P]),
                        in1=iota8[:, 0:npk * P]
                            .rearrange("p (c e) -> p c e", e=P),
                        op=AO.is_equal,
                    )
                    for i in range(npk):
                        cl = 8 * pk + i
                        h = cl // qcols
                        r = cl % qcols
                        bl = r // th
                        j = r % th
                        nc.tensor.matmul(
                            out=pss[bl],
                            lhsT=oh4[:, i * P:(i + 1) * P],
                            rhs=sqx[:, cl * w:(cl + 1) * w],
                            start=(h == 0 and j == 0),
                            stop=(h == NHALF - 1 and j == th - 1),
                        )

                # finishing: ACT copies PSUM->SBUF scaled by rz, then DVE
                # batched var over [P, gb*f], ACT sqrt, one DMA per group
                me = finp.tile([P, gb * w], F32, tag="me")
                m3 = me[:].rearrange("p (b e) -> p b e", e=w)
                for bl in range(gb):
                    b = g * gb + bl
                    nc.scalar.mul(
                        out=me[:, bl * w:(bl + 1) * w], in_=pss[bl],
                        mul=rz[:, b:b + 1])
                var = finp.tile([P, gb * f], F32, tag="var")
                v3 = var[:].rearrange("p (b e) -> p b e", e=f)
                nc.vector.tensor_tensor(
                    out=v3[:, :, :], in0=m3[:, :, 0:f], in1=m3[:, :, 0:f],
                    op=AO.mult)
                nc.vector.tensor_tensor(
                    out=v3[:, :, :], in0=m3[:, :, f:w], in1=v3[:, :, :],
                    op=AO.subtract)
                # clamp on ACT (not DVE tensor_scalar: single-src perf-mode
                # ops grab the DVE/GpSimd shared SBUF port and block against
                # long-running gather instructions holding it)
                std = ovp.tile([P, gb * f], F32, tag="std")
                nc.scalar.activation(out=var[:], in_=var[:],
                                     func=AF.Relu)
                nc.scalar.sqrt(out=std[:], in_=var[:])
                nc.sync.dma_start(
                    out=outd[g * P:(g + 1) * P, :], in_=std[:])
    return nc


def _pack_blocks(c0, c1):
    """Assign nodes to NBLK blocks of <=128 slots, balancing BOTH per-half
    in-edge sums toward <= 8*128 = 1024 (so the half tile capacity th is 8).
    Greedy on descending total degree, then swap refinement."""
    cap = 8 * P
    tot = c0 + c1
    order = np.argsort(-tot, kind="stable")
    l0 = np.zeros(NBLK)
    l1 = np.zeros(NBLK)
    ns = np.zeros(NBLK, np.int64)
    assign = np.empty(N_NODES, np.int64)
    for n in order:
        cost = np.maximum(l0 + c0[n], l1 + c1[n]) + 1e-3 * (l0 + l1)
        cost[ns >= P] = 1e18
        b = int(np.argmin(cost))
        assign[n] = b
        l0[b] += c0[n]
        l1[b] += c1[n]
        ns[b] += 1
    rng = np.random.default_rng(0)
    for _ in range(5000):
        over = np.maximum(l0 - cap, 0) + np.maximum(l1 - cap, 0)
        if over.sum() == 0:
            break
        b = int(np.argmax(over))
        half = 0 if l0[b] - cap >= l1[b] - cap else 1
        cb = c0 if half == 0 else c1
        members = np.nonzero(assign == b)[0]
        done = False
        for a in members[np.argsort(-cb[members])][:30]:
            cand = rng.integers(0, N_NODES, 8000)
            d = assign[cand]
            ok = ((l0[b] - c0[a] + c0[cand] <= cap)
                  & (l1[b] - c1[a] + c1[cand] <= cap)
                  & (l0[d] - c0[cand] + c0[a] <= cap)
                  & (l1[d] - c1[cand] + c1[a] <= cap) & (d != b))
            w = np.nonzero(ok)[0]
            if w.size:
                v = int(cand[w[0]])
                dd = assign[v]
                assign[a] = dd
                assign[v] = b
                l0[b] += c0[v] - c0[a]
                l1[b] += c1[v] - c1[a]
                l0[dd] += c0[a] - c0[v]
                l1[dd] += c1[a] - c1[v]
                done = True
                break
        if not done:
            break   # refinement stuck; th falls back to data max
    return assign


def _host_prep(x, edge_index):
    src = np.asarray(edge_index[0], dtype=np.int64)
    tgt = np.asarray(edge_index[1], dtype=np.int64)
    n_edges = src.shape[0]
    counts = np.bincount(tgt, minlength=N_NODES)

    c0 = np.bincount(tgt[src < NH], minlength=N_NODES)
    c1 = np.bincount(tgt[src >= NH], minlength=N_NODES)
    blk = _pack_blocks(c0, c1)
    # slot = index within block (stable by node id)
    order_b = np.argsort(blk, kind="stable")
    slot = np.empty(N_NODES, np.int64)
    bsort = blk[order_b]
    bstarts = np.zeros(NBLK, np.int64)
    np.cumsum(np.bincount(blk, minlength=NBLK)[:-1], out=bstarts[1:])
    slot[order_b] = np.arange(N_NODES) - bstarts[bsort]
    assert slot.max() < P

    eb = blk[tgt]                      # edge -> block
    eh = src // NH                     # edge -> src half
    es = slot[tgt]                     # edge -> slot in block
    seg = eb * NHALF + eh              # edge -> (block, half) segment
    segsums = np.bincount(seg, minlength=NBLK * NHALF)
    th = int(np.ceil(segsums.max() / P))
    cap = th * P

    # within each segment, order edges by src row (descending) for DRAM
    # gather locality; descending puts negative offset-idxs last so the
    # trailing-pop guard below can always find a swap partner
    order_e = np.lexsort((-src, seg))
    segs = seg[order_e]
    starts = np.zeros(NBLK * NHALF, np.int64)
    np.cumsum(segsums[:-1], out=starts[1:])
    within = np.arange(n_edges) - starts[segs]
    flat = segs * cap + within

    # idx values use the int16 offset trick: row r of half -> r - 32768;
    # padding slots use 0 (a valid row; one-hot column is all-zero)
    gidxq = np.zeros((NBLK, NHALF, cap), np.int16)
    tgtq = np.full((NBLK, NHALF, cap), -1.0, np.float32)
    gidxq.reshape(-1)[flat] = (src[order_e] % NH - IOFF).astype(np.int16)
    tgtq.reshape(-1)[flat] = es[order_e].astype(np.float32)

    # trailing-pop guard: the gather ucode drops trailing negative idxs from
    # each stream; ensure the final slot of every (core, group, half) stream
    # (= last block of the group, tile th-1, pos 127) has idx >= 0 by
    # swapping within its segment (edges may occupy any slot of their seg).
    # each (group, half) stream is gathered as two equal sub-gathers split at
    # column qcols//2; the ucode pops TRAILING negative idxs per sub-stream,
    # so each sub-stream's final slot must hold idx >= 0. Swap partners must
    # stay within the same segment AND at/before the boundary slot.
    ng = NB // GB
    qcols = GB * th
    for cb, cs in ((qcols // 2, 0), (qcols, qcols // 2)):
        end = cb * P - 1                    # flat slot in the (g,h) stream
        bl_end = end // cap                 # block within group
        pos = end % cap                     # position within segment
        lo = max(0, cs * P - bl_end * cap)  # earliest in-sub segment slot
        for c in range(NCORES):
            for g in range(ng):
                b = c * NB + g * GB + bl_end
                for h in range(NHALF):
                    if gidxq[b, h, pos] < 0 and tgtq[b, h, pos] >= 0:
                        cand = np.nonzero(gidxq[b, h, lo:pos + 1] >= 0)[0]
                        assert cand.size > 0, "no swap partner for pop guard"
                        jj = lo + cand[0]
                        gidxq[b, h, pos], gidxq[b, h, jj] = (
                            gidxq[b, h, jj], gidxq[b, h, pos])
                        tgtq[b, h, pos], tgtq[b, h, jj] = (
                            tgtq[b, h, jj], tgtq[b, h, pos])

    # packed per-node table [x | x^2] in bf16 (256B rows), padded for the
    # offset addressing window
    xf = np.asarray(x, dtype=np.float32)
    xpack = np.zeros((TROWS, 2 * N_FEAT), BF16)
    xpack[:N_NODES, :N_FEAT] = xf.astype(BF16)
    xpack[:N_NODES, N_FEAT:] = (xf * xf).astype(BF16)
    xpack = np.ascontiguousarray(xpack)

    # per-node (count>1)/max(count,eps), laid out [slot, block] per core
    rz_node = np.where(counts > 1, 1.0 / np.maximum(counts, EPS), 0.0)
    rz_node = rz_node.astype(np.float32)
    rz_all = np.zeros((NBLK, P), np.float32)
    rz_all[blk, slot] = rz_node
    rz_all = rz_all.reshape(NCORES, NB, P)

    i16c = GB * cap // 16

    in_maps = []
    for c in range(NCORES):
        tb = tgtq[c * NB:(c + 1) * NB]          # [NB, 2, cap]
        gi = gidxq[c * NB:(c + 1) * NB]
        # tgt columns: (group, half, block, tile) -> [P, C]
        tcore = (tb.reshape(ng, GB, NHALF, cap)
                 .transpose(0, 2, 1, 3)          # [ng, 2, GB, cap]
                 .reshape(ng * NHALF * GB * th, P).T)
        # idx16: per (group, half): stream of GB*cap idxs wrapped %16
        gs = (gi.reshape(ng, GB, NHALF, cap)
              .transpose(0, 2, 1, 3)             # [ng, 2, GB, cap]
              .reshape(ng * NHALF, GB * cap))    # per-gather streams
        idx16 = np.ascontiguousarray(
            np.tile(gs.reshape(ng * NHALF, i16c, 16).transpose(0, 2, 1)
                    .reshape(ng * NHALF * 16, i16c)
                    .reshape(ng * NHALF, 16, i16c)
                    .transpose(1, 0, 2).reshape(16, ng * NHALF * i16c),
                    (8, 1)))
        in_maps.append({
            "xpack": xpack,
            "gidx": idx16,
            "tgt": np.ascontiguousarray(tcore),
            "rz": np.ascontiguousarray(rz_all[c].T),   # [P, NB]
        })
    return th, in_maps, blk, slot


def _run(x, edge_index, trace=False):
    from concourse.bass_utils import run_bass_kernel_spmd

    th, in_maps, blk, slot = _host_prep(x, edge_index)
    key = ("prog", th)
    if key not in _CACHE:
        nc_ = _build_program(N_FEAT, NB, th, GB, NH)
        nc_.finalize()
        _CACHE[key] = nc_
    nc = _CACHE[key]
    res = run_bass_kernel_spmd(
        nc, in_maps, core_ids=list(range(NCORES)), trace=trace)

    # out layout: [ng*P, GB*f]; block b = g*GB + bl lives at rows g*P + slot,
    # cols bl*f:(bl+1)*f
    out_full = np.empty((N_NODES, N_FEAT), np.float32)
    ng = NB // GB
    cores = blk // NB
    for c in range(NCORES):
        o = np.asarray(res.results[c]["out"]).reshape(ng, P, GB, N_FEAT)
        m = cores == c
        bc = blk[m] % NB
        out_full[m] = o[bc // GB, slot[m], bc % GB]
    return out_full, res


def kernel(**inputs):
    out, _ = _run(inputs["x"], inputs["edge_index"], trace=False)
    return out
